# revision 25
# baseline (speedup 1.0000x reference)
"""Trainium2 Bass kernel for a basic multi-head attention layer.

Problem (hardcoded shapes):
  query/key/value: [4, 1024, 1024] f32, mask: [4, 1024, 1024] f32 (0/1)
  Wq/Wk/Wv: [1024, 1024] f32 (torch Linear layout [out, in]), biases [1024] f32
  out: [4, 1024, 1024] f32 = attention(q,k,v projections, mask), merged heads

Sharding: 8 cores; core c -> batch b = c//2, head-group g = c%2 (8 heads of 64).
Each core computes out[b][:, 512*g : 512*g+512]. No cross-core communication.

Device pipeline (per core), all layouts chosen so every DMA is contiguous:
  - projections run with the contraction (embed) dim on partitions, so the host
    ships X^T and W^T slices (bias folded in as one extra contraction row).
  - scores are computed transposed: S^T[k, q] = sum_d K^T[d,k] Q^T[d,q] (PSUM),
    two heads of a pair row-tiled into PE row-groups 0-63 / 64-127 (K=64 each)
  - ACT evacuates PSUM with exp() -> bf16 SBUF (no row-max subtraction needed:
    scores are O(5); the -1e6 additive mask is replaced by multiplying exp(s)
    with the 0/1 mask, exactly equivalent after normalization)
  - DVE multiplies by mask^T -> P^T
  - PV: out^T[m, q] = sum_k Vhat[k, m] P^T[k, q], where Vhat has a ones column
    appended so row 64 of out^T is the softmax denominator (free row-sum)
  - PE transposes out^T back to natural layout, DVE normalizes with reciprocal
    and writes final [q, 64] f32 stripes, DMA'd out per head.

Emission interleaves projection passes with attention blocks so the Scalar
engine (exp + PSUM evacuations) is busy from ~5us onward.
"""

import numpy as np
import ml_dtypes

import concourse.bass as bass
import concourse.mybir as mybir
import concourse.tile as tile
from concourse import bacc
from concourse.bass_utils import run_bass_kernel_spmd
from concourse.masks import make_identity

BF16 = mybir.dt.bfloat16
F32 = mybir.dt.float32
NP_BF16 = ml_dtypes.bfloat16

B, S, E = 4, 1024, 1024
H = 16  # total heads
D = 64  # head dim
HG = 8  # heads per core (head group)
GW = HG * D  # 512, output column width per core
N_CORES = 8
EC = E // 128  # 8 embed chunks
ST = S // 128  # 8 seq tiles
SCALE = float(D) ** -0.5

_PROG_CACHE = {}


def _build_program(reps: int = 1):
    nc = bacc.Bacc("TRN2")

    # All inputs are pre-arranged on the host so that a straight contiguous DMA
    # lands them in their SBUF layout ([128 partitions, ...free]).
    xq = nc.declare_dram_parameter("xq", [128, EC * S], BF16, isOutput=False)
    xk = nc.declare_dram_parameter("xk", [128, EC * S], BF16, isOutput=False)
    xv = nc.declare_dram_parameter("xv", [128, EC * S], BF16, isOutput=False)
    wq = nc.declare_dram_parameter("wq", [128, EC * GW], BF16, isOutput=False)
    wk = nc.declare_dram_parameter("wk", [128, EC * GW], BF16, isOutput=False)
    wv = nc.declare_dram_parameter("wv", [128, EC * GW], BF16, isOutput=False)
    bq = nc.declare_dram_parameter("bq", [1, GW], BF16, isOutput=False)
    bk = nc.declare_dram_parameter("bk", [1, GW], BF16, isOutput=False)
    bv = nc.declare_dram_parameter("bv", [1, GW], BF16, isOutput=False)
    maskt = nc.declare_dram_parameter("maskt", [128, ST * S], BF16, isOutput=False)
    out = nc.declare_dram_parameter("out", [S, GW], F32, isOutput=True)

    with tile.TileContext(nc) as tc:
        with (
            tc.tile_pool(name="persist", bufs=1) as persist,
            tc.tile_pool(name="xin", bufs=1) as xin,
            tc.tile_pool(name="win", bufs=1) as win,
            tc.tile_pool(name="pt", bufs=12) as ptp,
            tc.tile_pool(name="ot", bufs=2) as otp,
            tc.tile_pool(name="oh", bufs=2) as ohp,
            tc.tile_pool(name="rs", bufs=2) as rsp,
            tc.tile_pool(name="proj_ps", bufs=2, space="PSUM") as proj_ps,
            tc.tile_pool(name="sc_ps", bufs=2, space="PSUM") as sc_ps,
            tc.tile_pool(name="pv_ps", bufs=1, space="PSUM") as pv_ps,
            tc.tile_pool(name="xp_ps", bufs=1, space="PSUM") as xp_ps,
        ):
            bq_sb = persist.tile([1, GW], BF16)
            bk_sb = persist.tile([1, GW], BF16)
            bv_sb = persist.tile([1, GW], BF16)
            mask_sb = persist.tile([128, ST, S], BF16)
            qt_sb = persist.tile([128, H // 4, S], BF16)  # Q^T [d-in-pair, pair, q]
            kt_sb = persist.tile([128, H // 4, S], BF16)  # K^T [d-in-pair, pair, k]
            vhat_sb = persist.tile([128, ST, HG, D + 1], BF16)  # V + ones col
            xq_sb = xin.tile([128, EC, S], BF16, tag="xq")
            xk_sb = xin.tile([128, EC, S], BF16, tag="xk")
            xv_sb = xin.tile([128, EC, S], BF16, tag="xv")
            wq_sb = win.tile([128, EC, GW], BF16, tag="wq")
            wk_sb = win.tile([128, EC, GW], BF16, tag="wk")
            wv_sb = win.tile([128, EC, GW], BF16, tag="wv")

            def emit_loads():
                # loads in first-consumer order, chunked for early start
                for dst, src in ((bq_sb, bq), (bk_sb, bk), (bv_sb, bv)):
                    nc.sync.dma_start(out=dst, in_=src[:])
                for w_dst, w_src, x_dst, x_src in (
                    (wq_sb, wq, xq_sb, xq),
                    (wk_sb, wk, xk_sb, xk),
                ):
                    for ec in range(EC):
                        nc.sync.dma_start(out=w_dst[:, ec, :],
                                          in_=w_src[:, GW * ec:GW * ec + GW])
                        nc.sync.dma_start(out=x_dst[:, ec, :],
                                          in_=x_src[:, S * ec:S * ec + S])
                for kti in range(ST):
                    nc.sync.dma_start(out=mask_sb[:, kti, :],
                                      in_=maskt[:, S * kti:S * kti + S])
                for ec in range(EC):
                    nc.sync.dma_start(out=wv_sb[:, ec, :],
                                      in_=wv[:, GW * ec:GW * ec + GW])
                    nc.sync.dma_start(out=xv_sb[:, ec, :],
                                      in_=xv[:, S * ec:S * ec + S])

            ones_sb = persist.tile([1, S], BF16)
            nc.gpsimd.memset(ones_sb, 1.0)
            id_sb = persist.tile([128, 128], F32)
            make_identity(nc, id_sb)

            rep = 0  # current repetition (for unique tile names)

            def emit_qk_pass(w_sb, b_sb, x_sb, dst, dt_):
                """One [128, seq] stripe of the Q^T or K^T projection."""
                ps0 = proj_ps.tile([128, 512], F32, tag="pps",
                                   name=f"pps_{rep}_{dst.tensor.name}_{dt_}_0")
                ps1 = proj_ps.tile([128, 512], F32, tag="pps",
                                   name=f"pps_{rep}_{dst.tensor.name}_{dt_}_1")
                for ec in range(EC):
                    lhsT = w_sb[:, ec, 128 * dt_:128 * dt_ + 128]
                    nc.tensor.matmul(ps0, lhsT, x_sb[:, ec, 0:512],
                                     start=(ec == 0), stop=False)
                    nc.tensor.matmul(ps1, lhsT, x_sb[:, ec, 512:1024],
                                     start=(ec == 0), stop=False)
                bl = b_sb[0:1, 128 * dt_:128 * dt_ + 128]
                nc.tensor.matmul(ps0, bl, ones_sb[0:1, 0:512],
                                 start=False, stop=True)
                nc.tensor.matmul(ps1, bl, ones_sb[0:1, 512:1024],
                                 start=False, stop=True)
                # evacuate on the Scalar engine (idle otherwise; DVE is busy)
                nc.scalar.copy(dst[:, dt_, 0:512], ps0)
                nc.scalar.copy(dst[:, dt_, 512:1024], ps1)

            def emit_v_pass(p):
                """V projection (natural layout) for seq tiles 2p, 2p+1."""
                pss = [proj_ps.tile([128, 512], F32, tag="pps",
                                    name=f"pps_v_{rep}_{p}_{i}") for i in range(2)]
                for ec in range(EC):
                    for i in range(2):
                        st = 2 * p + i
                        nc.tensor.matmul(pss[i],
                                         xv_sb[:, ec, 128 * st:128 * st + 128],
                                         wv_sb[:, ec, :],
                                         start=(ec == 0), stop=False)
                for i in range(2):
                    st = 2 * p + i
                    nc.tensor.matmul(pss[i],
                                     ones_sb[0:1, 128 * st:128 * st + 128],
                                     bv_sb, start=False, stop=True)
                    nc.scalar.copy(
                        vhat_sb[:, st, :, 0:D],
                        pss[i].rearrange("p (h d) -> p h d", d=D),
                    )

            all_pts = {}

            def emit_scores(hp):
                pts = [ptp.tile([128, 2 * S], BF16, tag="pt",
                                name=f"pt_{rep}_{hp}_{kti}")
                       for kti in range(ST)]
                all_pts[hp] = pts
                for kti in range(ST):
                    # Row-tiled score matmuls: the two heads of the pair occupy
                    # PE row-groups 0-63 / 64-127 and run concurrently (K=64).
                    spss = []
                    for hl in range(2):
                        sps = sc_ps.tile([128, S], F32, tag="sps",
                                         name=f"sps_{rep}_{hp}_{kti}_{hl}")
                        spss.append(sps)
                        lhsT = kt_sb[64 * hl:64 * hl + 64, hp,
                                     128 * kti:128 * kti + 128]
                        rhs = qt_sb[64 * hl:64 * hl + 64, hp, :]
                        tp = (64 * hl, 0)
                        nc.tensor.matmul(sps[:, 0:512], lhsT, rhs[:, 0:512],
                                         tile_position=tp)
                        nc.tensor.matmul(sps[:, 512:1024], lhsT,
                                         rhs[:, 512:1024], tile_position=tp)
                    for hl in range(2):
                        nc.scalar.activation(
                            pts[kti][:, S * hl:S * hl + S], spss[hl],
                            mybir.ActivationFunctionType.Exp,
                        )
                    # multiply both heads' halves by the mask^T chunk
                    for hl in range(2):
                        half = pts[kti][:, S * hl:S * hl + S]
                        nc.vector.tensor_mul(half, half, mask_sb[:, kti, :])

            def emit_pv(hp):
                pts = all_pts[hp]
                for hl in range(2):
                    h = 2 * hp + hl
                    ot = otp.tile([D + 1, S], F32, tag="ot", name=f"ot_{rep}_{h}")
                    for qc in range(2):
                        pv = pv_ps.tile([D + 1, 512], F32, tag="pv",
                                        name=f"pv_{rep}_{h}_{qc}")
                        for kti in range(ST):
                            nc.tensor.matmul(
                                pv, vhat_sb[:, kti, h, :],
                                pts[kti][:, S * hl + 512 * qc:
                                         S * hl + 512 * qc + 512],
                                start=(kti == 0), stop=(kti == ST - 1))
                        nc.vector.tensor_copy(ot[:, 512 * qc:512 * qc + 512], pv)
                    oh = ohp.tile([128, ST, D], F32, tag="oh", name=f"oh_{rep}_{h}")
                    rs = rsp.tile([128, ST], F32, tag="rs", name=f"rs_{rep}_{h}")
                    for half in range(2):
                        # transpose back in 4-block groups (1 PSUM bank)
                        xp = xp_ps.tile([128, ST // 2, D + 1], F32, tag="xp",
                                        name=f"xp_{rep}_{h}_{half}")
                        for i in range(ST // 2):
                            t = 4 * half + i
                            nc.tensor.transpose(
                                xp[:, i, :], ot[:, 128 * t:128 * t + 128],
                                id_sb[0:D + 1, 0:D + 1],
                            )
                        nc.vector.reciprocal(rs[:, 4 * half:4 * half + 4],
                                             xp[:, :, D])
                        for i in range(ST // 2):
                            t = 4 * half + i
                            nc.vector.tensor_scalar_mul(
                                oh[:, t, :], xp[:, i, 0:D], rs[:, t:t + 1])
                    # stream this head's columns out now (hides the tail)
                    nc.sync.dma_start(
                        out=out[:, D * h:D * h + D].rearrange(
                            "(t p) d -> p t d", p=128),
                        in_=oh,
                    )

            # ---- interleaved emission: keeps ACT fed from ~5us onward ----
            for rep in range(reps):
                emit_loads()
                for hp in range(3):
                    emit_qk_pass(wq_sb, bq_sb, xq_sb, qt_sb, hp)
                    emit_qk_pass(wk_sb, bk_sb, xk_sb, kt_sb, hp)
                    emit_scores(hp)
                for p in range(4):
                    emit_v_pass(p)
                nc.gpsimd.memset(vhat_sb[:, :, :, D:D + 1], 1.0)
                emit_pv(0)
                emit_qk_pass(wq_sb, bq_sb, xq_sb, qt_sb, 3)
                emit_qk_pass(wk_sb, bk_sb, xk_sb, kt_sb, 3)
                emit_scores(3)
                emit_pv(1)
                emit_pv(2)
                emit_pv(3)

    nc.finalize()
    return nc


def _sbuf_chunks(a: np.ndarray) -> np.ndarray:
    """[C*128, F] -> [128, C*F] so partition p holds rows {128c + p} chunk-major."""
    c = a.shape[0] // 128
    return np.ascontiguousarray(
        a.reshape(c, 128, a.shape[1]).transpose(1, 0, 2).reshape(128, -1)
    )


def kernel(query, key, value, mask, Wq, bq, Wk, bk, Wv, bv):
    query = np.asarray(query, dtype=np.float32)
    key = np.asarray(key, dtype=np.float32)
    value = np.asarray(value, dtype=np.float32)
    mask = np.asarray(mask, dtype=np.float32)

    if "nc" not in _PROG_CACHE:
        _PROG_CACHE["nc"] = _build_program()
    nc = _PROG_CACHE["nc"]

    in_maps = []
    for c in range(N_CORES):
        b = c // 2
        g = c % 2
        sl = slice(GW * g, GW * g + GW)  # this core's head-group rows of W
        m = {}
        for name, x in (("xq", query[b]), ("xk", key[b]), ("xv", value[b])):
            m[name] = _sbuf_chunks(x.T.astype(NP_BF16))
        for name, w, scale in (("wq", Wq, SCALE), ("wk", Wk, 1.0), ("wv", Wv, 1.0)):
            wt = (np.asarray(w, dtype=np.float32)[sl] * scale).T  # [E, GW]
            m[name] = _sbuf_chunks(wt.astype(NP_BF16))
        for name, bias, scale in (("bq", bq, SCALE), ("bk", bk, 1.0), ("bv", bv, 1.0)):
            m[name] = (np.asarray(bias, dtype=np.float32)[sl] * scale).astype(
                NP_BF16).reshape(1, GW)
        m["maskt"] = _sbuf_chunks(np.ascontiguousarray(mask[b].T).astype(NP_BF16))
        in_maps.append(m)

    _PROG_CACHE["in_maps"] = in_maps
    res = run_bass_kernel_spmd(nc, in_maps, list(range(N_CORES)))
    outs = res.results

    full = np.empty((B, S, E), dtype=np.float32)
    for c in range(N_CORES):
        b = c // 2
        g = c % 2
        full[b, :, GW * g:GW * g + GW] = outs[c]["out"]
    return full


# revision 32
# speedup vs baseline: 86.0496x; 86.0496x over previous
"""Trainium2 Bass kernel for a basic multi-head attention layer.

Problem (hardcoded shapes):
  query/key/value: [4, 1024, 1024] f32, mask: [4, 1024, 1024] f32 (0/1)
  Wq/Wk/Wv: [1024, 1024] f32 (torch Linear layout [out, in]), biases [1024] f32
  out: [4, 1024, 1024] f32 = attention(q,k,v projections, mask), merged heads

Sharding: 8 cores; core c -> batch b = c//2, head-group g = c%2 (8 heads of 64).
Each core computes out[b][:, 512*g : 512*g+512]. No cross-core communication.

Device pipeline (per core), all layouts chosen so every DMA is contiguous:
  - projections run with the contraction (embed) dim on partitions, so the host
    ships X^T and W^T slices (bias folded in as one extra contraction row).
  - scores are computed transposed: S^T[k, q] = sum_d K^T[d,k] Q^T[d,q] (PSUM),
    two heads of a pair row-tiled into PE row-groups 0-63 / 64-127 (K=64 each)
  - ACT evacuates PSUM with exp() -> fp16 SBUF (no row-max subtraction needed:
    scores are O(5); the -1e6 additive mask is replaced by multiplying exp(s)
    with the 0/1 mask, exactly equivalent after normalization)
  - DVE multiplies by mask^T -> P^T
  - PV: out^T[m, q] = sum_k Vhat[k, m] P^T[k, q], where Vhat has a ones column
    appended so row 64 of out^T is the softmax denominator (free row-sum)
  - PE transposes out^T back to natural layout, DVE normalizes with reciprocal
    and writes final [q, 64] f32 stripes, DMA'd out per head.

Emission interleaves projection passes with attention blocks so the Scalar
engine (exp + PSUM evacuations) is busy from ~5us onward.
"""

import numpy as np

import concourse.bass as bass
import concourse.mybir as mybir
import concourse.tile as tile
from concourse import bacc
from concourse.bass_utils import run_bass_kernel_spmd
from concourse.masks import make_identity

# fp16 over bf16: same PE/DVE throughput class, ~16x less rounding error,
# and every tensor here fits fp16 range (|x|,|q|,|k|,|v| ~ O(5), exp(s) <= ~400)
DT16 = mybir.dt.float16
F32 = mybir.dt.float32
NP16 = np.float16

B, S, E = 4, 1024, 1024
H = 16  # total heads
D = 64  # head dim
HG = 8  # heads per core (head group)
GW = HG * D  # 512, output column width per core
N_CORES = 8
EC = E // 128  # 8 embed chunks
ST = S // 128  # 8 seq tiles
SCALE = float(D) ** -0.5

_PROG_CACHE = {}


def _build_program(reps: int = 1, row_tile: bool = True):
    nc = bacc.Bacc("TRN2")

    # All inputs are pre-arranged on the host so that a straight contiguous DMA
    # lands them in their SBUF layout ([128 partitions, ...free]).
    xq = nc.declare_dram_parameter("xq", [128, EC * S], DT16, isOutput=False)
    xk = nc.declare_dram_parameter("xk", [128, EC * S], DT16, isOutput=False)
    xv = nc.declare_dram_parameter("xv", [128, EC * S], DT16, isOutput=False)
    wq = nc.declare_dram_parameter("wq", [128, EC * GW], DT16, isOutput=False)
    wk = nc.declare_dram_parameter("wk", [128, EC * GW], DT16, isOutput=False)
    wv = nc.declare_dram_parameter("wv", [128, EC * GW], DT16, isOutput=False)
    bq = nc.declare_dram_parameter("bq", [1, GW], DT16, isOutput=False)
    bk = nc.declare_dram_parameter("bk", [1, GW], DT16, isOutput=False)
    bv = nc.declare_dram_parameter("bv", [1, GW], DT16, isOutput=False)
    maskt = nc.declare_dram_parameter("maskt", [128, ST * S], DT16, isOutput=False)
    out = nc.declare_dram_parameter("out", [S, GW], F32, isOutput=True)

    with tile.TileContext(nc) as tc:
        with (
            tc.tile_pool(name="persist", bufs=1) as persist,
            tc.tile_pool(name="xin", bufs=1) as xin,
            tc.tile_pool(name="win", bufs=1) as win,
            tc.tile_pool(name="pt", bufs=12) as ptp,
            tc.tile_pool(name="ot", bufs=2) as otp,
            tc.tile_pool(name="oh", bufs=2) as ohp,
            tc.tile_pool(name="rs", bufs=2) as rsp,
            tc.tile_pool(name="proj_ps", bufs=2, space="PSUM") as proj_ps,
            tc.tile_pool(name="sc_ps", bufs=2, space="PSUM") as sc_ps,
            tc.tile_pool(name="pv_ps", bufs=1, space="PSUM") as pv_ps,
            tc.tile_pool(name="xp_ps", bufs=1, space="PSUM") as xp_ps,
        ):
            bq_sb = persist.tile([1, GW], DT16)
            bk_sb = persist.tile([1, GW], DT16)
            bv_sb = persist.tile([1, GW], DT16)
            mask_sb = persist.tile([128, ST, S], DT16)
            qt_sb = persist.tile([128, H // 4, S], DT16)  # Q^T [d-in-pair, pair, q]
            kt_sb = persist.tile([128, H // 4, S], DT16)  # K^T [d-in-pair, pair, k]
            vhat_sb = persist.tile([128, ST, HG, D + 1], DT16)  # V + ones col
            xq_sb = xin.tile([128, EC, S], DT16, tag="xq")
            xk_sb = xin.tile([128, EC, S], DT16, tag="xk")
            xv_sb = xin.tile([128, EC, S], DT16, tag="xv")
            wq_sb = win.tile([128, EC, GW], DT16, tag="wq")
            wk_sb = win.tile([128, EC, GW], DT16, tag="wk")
            wv_sb = win.tile([128, EC, GW], DT16, tag="wv")

            def emit_loads():
                # loads in first-consumer order; xq/xk interleaved in 2-chunk
                # pieces so the Q and K projections pace together off the DMA
                # stream without paying per-DMA fixed cost 59 times
                for dst, src in ((bq_sb, bq), (bk_sb, bk), (bv_sb, bv)):
                    nc.sync.dma_start(out=dst, in_=src[:])
                nc.sync.dma_start(out=wq_sb, in_=wq[:].rearrange(
                    "p (a b) -> p a b", b=GW))
                nc.sync.dma_start(out=wk_sb, in_=wk[:].rearrange(
                    "p (a b) -> p a b", b=GW))
                for c in range(EC // 2):
                    nc.sync.dma_start(
                        out=xq_sb[:, 2 * c:2 * c + 2, :],
                        in_=xq[:, 2 * S * c:2 * S * c + 2 * S].rearrange(
                            "p (a b) -> p a b", b=S))
                    nc.sync.dma_start(
                        out=xk_sb[:, 2 * c:2 * c + 2, :],
                        in_=xk[:, 2 * S * c:2 * S * c + 2 * S].rearrange(
                            "p (a b) -> p a b", b=S))
                for c in range(2):
                    nc.sync.dma_start(
                        out=mask_sb[:, 4 * c:4 * c + 4, :],
                        in_=maskt[:, 4 * S * c:4 * S * c + 4 * S].rearrange(
                            "p (a b) -> p a b", b=S))
                nc.sync.dma_start(out=wv_sb, in_=wv[:].rearrange(
                    "p (a b) -> p a b", b=GW))
                for c in range(EC // 2):
                    nc.sync.dma_start(
                        out=xv_sb[:, 2 * c:2 * c + 2, :],
                        in_=xv[:, 2 * S * c:2 * S * c + 2 * S].rearrange(
                            "p (a b) -> p a b", b=S))

            ones_sb = persist.tile([1, S], DT16)
            nc.gpsimd.memset(ones_sb, 1.0)
            id_sb = persist.tile([128, 128], F32)
            make_identity(nc, id_sb)

            rep = 0  # current repetition (for unique tile names)

            def emit_qk_pass(w_sb, b_sb, x_sb, dst, dt_):
                """One [128, seq] stripe of the Q^T or K^T projection."""
                ps0 = proj_ps.tile([128, 512], F32, tag="pps",
                                   name=f"pps_{rep}_{dst.tensor.name}_{dt_}_0")
                ps1 = proj_ps.tile([128, 512], F32, tag="pps",
                                   name=f"pps_{rep}_{dst.tensor.name}_{dt_}_1")
                for ec in range(EC):
                    lhsT = w_sb[:, ec, 128 * dt_:128 * dt_ + 128]
                    nc.tensor.matmul(ps0, lhsT, x_sb[:, ec, 0:512],
                                     start=(ec == 0), stop=False)
                    nc.tensor.matmul(ps1, lhsT, x_sb[:, ec, 512:1024],
                                     start=(ec == 0), stop=False)
                bl = b_sb[0:1, 128 * dt_:128 * dt_ + 128]
                nc.tensor.matmul(ps0, bl, ones_sb[0:1, 0:512],
                                 start=False, stop=True)
                nc.tensor.matmul(ps1, bl, ones_sb[0:1, 512:1024],
                                 start=False, stop=True)
                # evacuate on the Scalar engine (idle otherwise; DVE is busy)
                nc.scalar.copy(dst[:, dt_, 0:512], ps0)
                nc.scalar.copy(dst[:, dt_, 512:1024], ps1)

            def emit_v_pass(p):
                """V projection (natural layout) for seq tiles 2p, 2p+1."""
                pss = [proj_ps.tile([128, 512], F32, tag="pps",
                                    name=f"pps_v_{rep}_{p}_{i}") for i in range(2)]
                for ec in range(EC):
                    for i in range(2):
                        st = 2 * p + i
                        nc.tensor.matmul(pss[i],
                                         xv_sb[:, ec, 128 * st:128 * st + 128],
                                         wv_sb[:, ec, :],
                                         start=(ec == 0), stop=False)
                for i in range(2):
                    st = 2 * p + i
                    nc.tensor.matmul(pss[i],
                                     ones_sb[0:1, 128 * st:128 * st + 128],
                                     bv_sb, start=False, stop=True)
                    nc.scalar.copy(
                        vhat_sb[:, st, :, 0:D],
                        pss[i].rearrange("p (h d) -> p h d", d=D),
                    )

            all_pts = {}

            def emit_scores(hp):
                pts = [ptp.tile([128, 2 * S], DT16, tag="pt",
                                name=f"pt_{rep}_{hp}_{kti}")
                       for kti in range(ST)]
                all_pts[hp] = pts
                for kti in range(ST):
                    # Row-tiled score matmuls: the two heads of the pair occupy
                    # PE row-groups 0-63 / 64-127 and run concurrently (K=64).
                    spss = []
                    for hl in range(2):
                        sps = sc_ps.tile([128, S], F32, tag="sps",
                                         name=f"sps_{rep}_{hp}_{kti}_{hl}")
                        spss.append(sps)
                        lhsT = kt_sb[64 * hl:64 * hl + 64, hp,
                                     128 * kti:128 * kti + 128]
                        rhs = qt_sb[64 * hl:64 * hl + 64, hp, :]
                        tp = (64 * hl, 0) if row_tile else None
                        nc.tensor.matmul(sps[:, 0:512], lhsT, rhs[:, 0:512],
                                         tile_position=tp)
                        nc.tensor.matmul(sps[:, 512:1024], lhsT,
                                         rhs[:, 512:1024], tile_position=tp)
                    for hl in range(2):
                        nc.scalar.activation(
                            pts[kti][:, S * hl:S * hl + S], spss[hl],
                            mybir.ActivationFunctionType.Exp,
                        )
                    # multiply both heads' halves by the mask^T chunk
                    for hl in range(2):
                        half = pts[kti][:, S * hl:S * hl + S]
                        nc.vector.tensor_mul(half, half, mask_sb[:, kti, :])

            def emit_pv(hp):
                pts = all_pts[hp]
                for hl in range(2):
                    h = 2 * hp + hl
                    ot = otp.tile([D + 1, S], F32, tag="ot", name=f"ot_{rep}_{h}")
                    for qc in range(2):
                        pv = pv_ps.tile([D + 1, 512], F32, tag="pv",
                                        name=f"pv_{rep}_{h}_{qc}")
                        for kti in range(ST):
                            nc.tensor.matmul(
                                pv, vhat_sb[:, kti, h, :],
                                pts[kti][:, S * hl + 512 * qc:
                                         S * hl + 512 * qc + 512],
                                start=(kti == 0), stop=(kti == ST - 1))
                        nc.vector.tensor_copy(ot[:, 512 * qc:512 * qc + 512], pv)
                    oh = ohp.tile([128, ST, D], F32, tag="oh", name=f"oh_{rep}_{h}")
                    rs = rsp.tile([128, ST], F32, tag="rs", name=f"rs_{rep}_{h}")
                    for half in range(2):
                        # transpose back in 4-block groups (1 PSUM bank)
                        xp = xp_ps.tile([128, ST // 2, D + 1], F32, tag="xp",
                                        name=f"xp_{rep}_{h}_{half}")
                        for i in range(ST // 2):
                            t = 4 * half + i
                            nc.tensor.transpose(
                                xp[:, i, :], ot[:, 128 * t:128 * t + 128],
                                id_sb[0:D + 1, 0:D + 1],
                            )
                        nc.vector.reciprocal(rs[:, 4 * half:4 * half + 4],
                                             xp[:, :, D])
                        for i in range(ST // 2):
                            t = 4 * half + i
                            nc.vector.tensor_scalar_mul(
                                oh[:, t, :], xp[:, i, 0:D], rs[:, t:t + 1])
                    # stream this head's columns out now (hides the tail)
                    nc.sync.dma_start(
                        out=out[:, D * h:D * h + D].rearrange(
                            "(t p) d -> p t d", p=128),
                        in_=oh,
                    )

            # ---- interleaved emission: keeps ACT fed from ~5us onward ----
            for rep in range(reps):
                emit_loads()
                for hp in range(3):
                    emit_qk_pass(wq_sb, bq_sb, xq_sb, qt_sb, hp)
                    emit_qk_pass(wk_sb, bk_sb, xk_sb, kt_sb, hp)
                    emit_scores(hp)
                for p in range(4):
                    emit_v_pass(p)
                nc.gpsimd.memset(vhat_sb[:, :, :, D:D + 1], 1.0)
                emit_pv(0)
                emit_qk_pass(wq_sb, bq_sb, xq_sb, qt_sb, 3)
                emit_qk_pass(wk_sb, bk_sb, xk_sb, kt_sb, 3)
                emit_scores(3)
                emit_pv(1)
                emit_pv(2)
                emit_pv(3)

    nc.finalize()
    return nc


def _sbuf_chunks(a: np.ndarray) -> np.ndarray:
    """[C*128, F] -> [128, C*F] so partition p holds rows {128c + p} chunk-major."""
    c = a.shape[0] // 128
    return np.ascontiguousarray(
        a.reshape(c, 128, a.shape[1]).transpose(1, 0, 2).reshape(128, -1)
    )


def kernel(query, key, value, mask, Wq, bq, Wk, bk, Wv, bv):
    query = np.asarray(query, dtype=np.float32)
    key = np.asarray(key, dtype=np.float32)
    value = np.asarray(value, dtype=np.float32)
    mask = np.asarray(mask, dtype=np.float32)

    if "nc" not in _PROG_CACHE:
        _PROG_CACHE["nc"] = _build_program()
    nc = _PROG_CACHE["nc"]

    in_maps = []
    for c in range(N_CORES):
        b = c // 2
        g = c % 2
        sl = slice(GW * g, GW * g + GW)  # this core's head-group rows of W
        m = {}
        for name, x in (("xq", query[b]), ("xk", key[b]), ("xv", value[b])):
            m[name] = _sbuf_chunks(x.T.astype(NP16))
        for name, w, scale in (("wq", Wq, SCALE), ("wk", Wk, 1.0), ("wv", Wv, 1.0)):
            wt = (np.asarray(w, dtype=np.float32)[sl] * scale).T  # [E, GW]
            m[name] = _sbuf_chunks(wt.astype(NP16))
        for name, bias, scale in (("bq", bq, SCALE), ("bk", bk, 1.0), ("bv", bv, 1.0)):
            m[name] = (np.asarray(bias, dtype=np.float32)[sl] * scale).astype(
                NP16).reshape(1, GW)
        m["maskt"] = _sbuf_chunks(np.ascontiguousarray(mask[b].T).astype(NP16))
        in_maps.append(m)

    _PROG_CACHE["in_maps"] = in_maps
    res = run_bass_kernel_spmd(nc, in_maps, list(range(N_CORES)))
    outs = res.results

    full = np.empty((B, S, E), dtype=np.float32)
    for c in range(N_CORES):
        b = c // 2
        g = c % 2
        full[b, :, GW * g:GW * g + GW] = outs[c]["out"]
    return full


# revision 35
# speedup vs baseline: 91.6770x; 1.0654x over previous
"""Trainium2 Bass kernel for a basic multi-head attention layer.

Problem (hardcoded shapes):
  query/key/value: [4, 1024, 1024] f32, mask: [4, 1024, 1024] f32 (0/1)
  Wq/Wk/Wv: [1024, 1024] f32 (torch Linear layout [out, in]), biases [1024] f32
  out: [4, 1024, 1024] f32 = attention(q,k,v projections, mask), merged heads

Sharding: 8 cores; core c -> batch b = c//2, head-group g = c%2 (8 heads of 64).
Each core computes out[b][:, 512*g : 512*g+512]. No cross-core communication.

Device pipeline (per core), all layouts chosen so every DMA is contiguous:
  - projections run with the contraction (embed) dim on partitions, so the host
    ships X^T and W^T slices (bias folded in as one extra contraction row).
  - scores are computed transposed: S^T[k, q] = sum_d K^T[d,k] Q^T[d,q] (PSUM),
    two heads of a pair row-tiled into PE row-groups 0-63 / 64-127 (K=64 each)
  - ACT evacuates PSUM with exp() -> fp16 SBUF (no row-max subtraction needed:
    scores are O(5); the -1e6 additive mask is replaced by multiplying exp(s)
    with the 0/1 mask, exactly equivalent after normalization)
  - DVE multiplies by mask^T -> P^T
  - PV: out^T[m, q] = sum_k Vhat[k, m] P^T[k, q], where Vhat has a ones column
    appended so row 64 of out^T is the softmax denominator (free row-sum)
  - PE transposes out^T back to natural layout, DVE normalizes with reciprocal
    and writes final [q, 64] f32 stripes, DMA'd out per head.

Emission interleaves projection passes with attention blocks so the Scalar
engine (exp + PSUM evacuations) is busy from ~5us onward.
"""

import numpy as np

import concourse.bass as bass
import concourse.mybir as mybir
import concourse.tile as tile
from concourse import bacc
from concourse.bass_utils import run_bass_kernel_spmd
from concourse.masks import make_identity

# fp16 over bf16: same PE/DVE throughput class, ~16x less rounding error,
# and every tensor here fits fp16 range (|x|,|q|,|k|,|v| ~ O(5), exp(s) <= ~400)
DT16 = mybir.dt.float16
F32 = mybir.dt.float32
NP16 = np.float16

B, S, E = 4, 1024, 1024
H = 16  # total heads
D = 64  # head dim
HG = 8  # heads per core (head group)
GW = HG * D  # 512, output column width per core
N_CORES = 8
EC = E // 128  # 8 embed chunks
ST = S // 128  # 8 seq tiles
SCALE = float(D) ** -0.5

_PROG_CACHE = {}


def _build_program(reps: int = 1, row_tile: bool = True):
    nc = bacc.Bacc("TRN2")

    # All inputs are pre-arranged on the host so that a straight contiguous DMA
    # lands them in their SBUF layout ([128 partitions, ...free]).
    xq = nc.declare_dram_parameter("xq", [128, EC * S], DT16, isOutput=False)
    xk = nc.declare_dram_parameter("xk", [128, EC * S], DT16, isOutput=False)
    xv = nc.declare_dram_parameter("xv", [128, EC * S], DT16, isOutput=False)
    wq = nc.declare_dram_parameter("wq", [128, EC * GW], DT16, isOutput=False)
    wk = nc.declare_dram_parameter("wk", [128, EC * GW], DT16, isOutput=False)
    wv = nc.declare_dram_parameter("wv", [128, EC * GW], DT16, isOutput=False)
    bq = nc.declare_dram_parameter("bq", [1, GW], DT16, isOutput=False)
    bk = nc.declare_dram_parameter("bk", [1, GW], DT16, isOutput=False)
    bv = nc.declare_dram_parameter("bv", [1, GW], DT16, isOutput=False)
    maskt = nc.declare_dram_parameter("maskt", [128, ST * S], DT16, isOutput=False)
    out = nc.declare_dram_parameter("out", [S, GW], F32, isOutput=True)

    with tile.TileContext(nc) as tc:
        with (
            tc.tile_pool(name="persist", bufs=1) as persist,
            tc.tile_pool(name="xin", bufs=1) as xin,
            tc.tile_pool(name="win", bufs=1) as win,
            tc.tile_pool(name="pt", bufs=12) as ptp,
            tc.tile_pool(name="ot", bufs=2) as otp,
            tc.tile_pool(name="oh", bufs=2) as ohp,
            tc.tile_pool(name="rs", bufs=2) as rsp,
            tc.tile_pool(name="proj_ps", bufs=2, space="PSUM") as proj_ps,
            tc.tile_pool(name="sc_ps", bufs=2, space="PSUM") as sc_ps,
            tc.tile_pool(name="pv_ps", bufs=1, space="PSUM") as pv_ps,
            tc.tile_pool(name="xp_ps", bufs=1, space="PSUM") as xp_ps,
        ):
            bq_sb = persist.tile([1, GW], DT16)
            bk_sb = persist.tile([1, GW], DT16)
            bv_sb = persist.tile([1, GW], DT16)
            mask_sb = persist.tile([128, ST, S], DT16)
            qt_sb = persist.tile([128, H // 4, S], DT16)  # Q^T [d-in-pair, pair, q]
            kt_sb = persist.tile([128, H // 4, S], DT16)  # K^T [d-in-pair, pair, k]
            vhat_sb = persist.tile([128, ST, HG, D + 1], DT16)  # V + ones col
            xq_sb = xin.tile([128, EC, S], DT16, tag="xq")
            xk_sb = xin.tile([128, EC, S], DT16, tag="xk")
            xv_sb = xin.tile([128, EC, S], DT16, tag="xv")
            wq_sb = win.tile([128, EC, GW], DT16, tag="wq")
            wk_sb = win.tile([128, EC, GW], DT16, tag="wk")
            wv_sb = win.tile([128, EC, GW], DT16, tag="wv")

            def emit_loads():
                # loads in first-consumer order; xq/xk interleaved in 2-chunk
                # pieces so the Q and K projections pace together off the DMA
                # stream without paying per-DMA fixed cost 59 times
                for dst, src in ((bq_sb, bq), (bk_sb, bk), (bv_sb, bv)):
                    nc.sync.dma_start(out=dst, in_=src[:])
                nc.sync.dma_start(out=wq_sb, in_=wq[:].rearrange(
                    "p (a b) -> p a b", b=GW))
                nc.sync.dma_start(out=wk_sb, in_=wk[:].rearrange(
                    "p (a b) -> p a b", b=GW))
                for c in range(EC // 2):
                    nc.sync.dma_start(
                        out=xq_sb[:, 2 * c:2 * c + 2, :],
                        in_=xq[:, 2 * S * c:2 * S * c + 2 * S].rearrange(
                            "p (a b) -> p a b", b=S))
                    nc.sync.dma_start(
                        out=xk_sb[:, 2 * c:2 * c + 2, :],
                        in_=xk[:, 2 * S * c:2 * S * c + 2 * S].rearrange(
                            "p (a b) -> p a b", b=S))
                for c in range(2):
                    nc.sync.dma_start(
                        out=mask_sb[:, 4 * c:4 * c + 4, :],
                        in_=maskt[:, 4 * S * c:4 * S * c + 4 * S].rearrange(
                            "p (a b) -> p a b", b=S))
                nc.sync.dma_start(out=wv_sb, in_=wv[:].rearrange(
                    "p (a b) -> p a b", b=GW))
                for c in range(EC // 2):
                    nc.sync.dma_start(
                        out=xv_sb[:, 2 * c:2 * c + 2, :],
                        in_=xv[:, 2 * S * c:2 * S * c + 2 * S].rearrange(
                            "p (a b) -> p a b", b=S))

            ones_sb = persist.tile([1, S], DT16)
            nc.gpsimd.memset(ones_sb, 1.0)
            id_sb = persist.tile([128, 128], F32)
            make_identity(nc, id_sb)

            rep = 0  # current repetition (for unique tile names)

            def emit_qk_pass(w_sb, b_sb, x_sb, dst, dt_):
                """One [128, seq] stripe of the Q^T or K^T projection.
                Generator: yields after each embed chunk (2 matmuls) so score
                blocks can pull projection work into their PE stalls."""
                ps0 = proj_ps.tile([128, 512], F32, tag="pps",
                                   name=f"pps_{rep}_{dst.tensor.name}_{dt_}_0")
                ps1 = proj_ps.tile([128, 512], F32, tag="pps",
                                   name=f"pps_{rep}_{dst.tensor.name}_{dt_}_1")
                for ec in range(EC):
                    lhsT = w_sb[:, ec, 128 * dt_:128 * dt_ + 128]
                    nc.tensor.matmul(ps0, lhsT, x_sb[:, ec, 0:512],
                                     start=(ec == 0), stop=False)
                    nc.tensor.matmul(ps1, lhsT, x_sb[:, ec, 512:1024],
                                     start=(ec == 0), stop=False)
                    yield
                bl = b_sb[0:1, 128 * dt_:128 * dt_ + 128]
                nc.tensor.matmul(ps0, bl, ones_sb[0:1, 0:512],
                                 start=False, stop=True)
                nc.tensor.matmul(ps1, bl, ones_sb[0:1, 512:1024],
                                 start=False, stop=True)
                # evacuate on the Scalar engine (idle otherwise; DVE is busy)
                nc.scalar.copy(dst[:, dt_, 0:512], ps0)
                nc.scalar.copy(dst[:, dt_, 512:1024], ps1)

            def emit_v_pass(p):
                """V projection (natural layout) for seq tiles 2p, 2p+1.
                Generator, same contract as emit_qk_pass."""
                pss = [proj_ps.tile([128, 512], F32, tag="pps",
                                    name=f"pps_v_{rep}_{p}_{i}") for i in range(2)]
                for ec in range(EC):
                    for i in range(2):
                        st = 2 * p + i
                        nc.tensor.matmul(pss[i],
                                         xv_sb[:, ec, 128 * st:128 * st + 128],
                                         wv_sb[:, ec, :],
                                         start=(ec == 0), stop=False)
                    yield
                for i in range(2):
                    st = 2 * p + i
                    nc.tensor.matmul(pss[i],
                                     ones_sb[0:1, 128 * st:128 * st + 128],
                                     bv_sb, start=False, stop=True)
                    nc.scalar.copy(
                        vhat_sb[:, st, :, 0:D],
                        pss[i].rearrange("p (h d) -> p h d", d=D),
                    )

            def drain(gen):
                for _ in gen:
                    pass

            all_pts = {}

            def emit_scores(hp, filler=(), pulls=0):
                pts = [ptp.tile([128, 2 * S], DT16, tag="pt",
                                name=f"pt_{rep}_{hp}_{kti}")
                       for kti in range(ST)]
                all_pts[hp] = pts
                filler = iter(filler)
                for kti in range(ST):
                    # Row-tiled score matmuls: the two heads of the pair occupy
                    # PE row-groups 0-63 / 64-127 and run concurrently (K=64).
                    spss = []
                    for hl in range(2):
                        sps = sc_ps.tile([128, S], F32, tag="sps",
                                         name=f"sps_{rep}_{hp}_{kti}_{hl}")
                        spss.append(sps)
                        lhsT = kt_sb[64 * hl:64 * hl + 64, hp,
                                     128 * kti:128 * kti + 128]
                        rhs = qt_sb[64 * hl:64 * hl + 64, hp, :]
                        tp = (64 * hl, 0) if row_tile else None
                        nc.tensor.matmul(sps[:, 0:512], lhsT, rhs[:, 0:512],
                                         tile_position=tp)
                        nc.tensor.matmul(sps[:, 512:1024], lhsT,
                                         rhs[:, 512:1024], tile_position=tp)
                    for hl in range(2):
                        nc.scalar.activation(
                            pts[kti][:, S * hl:S * hl + S], spss[hl],
                            mybir.ActivationFunctionType.Exp,
                        )
                    # multiply both heads' halves by the mask^T chunk
                    for hl in range(2):
                        half = pts[kti][:, S * hl:S * hl + S]
                        nc.vector.tensor_mul(half, half, mask_sb[:, kti, :])
                    # interleave a slice of the next projection pass so the
                    # (in-order) PE has work while ACT drains this kti's exps
                    for _ in range(pulls):
                        next(filler, None)
                drain(filler)

            def emit_pv(hp):
                pts = all_pts[hp]
                for hl in range(2):
                    h = 2 * hp + hl
                    ot = otp.tile([D + 1, S], F32, tag="ot", name=f"ot_{rep}_{h}")
                    for qc in range(2):
                        pv = pv_ps.tile([D + 1, 512], F32, tag="pv",
                                        name=f"pv_{rep}_{h}_{qc}")
                        for kti in range(ST):
                            nc.tensor.matmul(
                                pv, vhat_sb[:, kti, h, :],
                                pts[kti][:, S * hl + 512 * qc:
                                         S * hl + 512 * qc + 512],
                                start=(kti == 0), stop=(kti == ST - 1))
                        nc.vector.tensor_copy(ot[:, 512 * qc:512 * qc + 512], pv)
                    oh = ohp.tile([128, ST, D], F32, tag="oh", name=f"oh_{rep}_{h}")
                    rs = rsp.tile([128, ST], F32, tag="rs", name=f"rs_{rep}_{h}")
                    for half in range(2):
                        # transpose back in 4-block groups (1 PSUM bank)
                        xp = xp_ps.tile([128, ST // 2, D + 1], F32, tag="xp",
                                        name=f"xp_{rep}_{h}_{half}")
                        for i in range(ST // 2):
                            t = 4 * half + i
                            nc.tensor.transpose(
                                xp[:, i, :], ot[:, 128 * t:128 * t + 128],
                                id_sb[0:D + 1, 0:D + 1],
                            )
                        nc.vector.reciprocal(rs[:, 4 * half:4 * half + 4],
                                             xp[:, :, D])
                        for i in range(ST // 2):
                            t = 4 * half + i
                            nc.vector.tensor_scalar_mul(
                                oh[:, t, :], xp[:, i, 0:D], rs[:, t:t + 1])
                    # stream this head's columns out now (hides the tail)
                    nc.sync.dma_start(
                        out=out[:, D * h:D * h + D].rearrange(
                            "(t p) d -> p t d", p=128),
                        in_=oh,
                    )

            # ---- interleaved emission: keeps ACT fed from ~5us onward and
            # fills the PE's in-order stalls inside score blocks with the
            # next projection pass's matmuls ----
            from itertools import chain

            for rep in range(reps):
                emit_loads()
                drain(emit_qk_pass(wq_sb, bq_sb, xq_sb, qt_sb, 0))
                drain(emit_qk_pass(wk_sb, bk_sb, xk_sb, kt_sb, 0))
                nc.gpsimd.memset(vhat_sb[:, :, :, D:D + 1], 1.0)
                emit_scores(0, chain(
                    emit_qk_pass(wq_sb, bq_sb, xq_sb, qt_sb, 1),
                    emit_qk_pass(wk_sb, bk_sb, xk_sb, kt_sb, 1)), pulls=2)
                emit_scores(1, chain(
                    emit_qk_pass(wq_sb, bq_sb, xq_sb, qt_sb, 2),
                    emit_qk_pass(wk_sb, bk_sb, xk_sb, kt_sb, 2),
                    *[emit_v_pass(p) for p in range(4)]), pulls=6)
                emit_pv(0)
                emit_scores(2, chain(
                    emit_qk_pass(wq_sb, bq_sb, xq_sb, qt_sb, 3),
                    emit_qk_pass(wk_sb, bk_sb, xk_sb, kt_sb, 3)), pulls=2)
                emit_pv(1)
                emit_scores(3)
                emit_pv(2)
                emit_pv(3)

    nc.finalize()
    return nc


def _sbuf_chunks(a: np.ndarray) -> np.ndarray:
    """[C*128, F] -> [128, C*F] so partition p holds rows {128c + p} chunk-major."""
    c = a.shape[0] // 128
    return np.ascontiguousarray(
        a.reshape(c, 128, a.shape[1]).transpose(1, 0, 2).reshape(128, -1)
    )


def kernel(query, key, value, mask, Wq, bq, Wk, bk, Wv, bv):
    query = np.asarray(query, dtype=np.float32)
    key = np.asarray(key, dtype=np.float32)
    value = np.asarray(value, dtype=np.float32)
    mask = np.asarray(mask, dtype=np.float32)

    if "nc" not in _PROG_CACHE:
        _PROG_CACHE["nc"] = _build_program()
    nc = _PROG_CACHE["nc"]

    in_maps = []
    for c in range(N_CORES):
        b = c // 2
        g = c % 2
        sl = slice(GW * g, GW * g + GW)  # this core's head-group rows of W
        m = {}
        for name, x in (("xq", query[b]), ("xk", key[b]), ("xv", value[b])):
            m[name] = _sbuf_chunks(x.T.astype(NP16))
        for name, w, scale in (("wq", Wq, SCALE), ("wk", Wk, 1.0), ("wv", Wv, 1.0)):
            wt = (np.asarray(w, dtype=np.float32)[sl] * scale).T  # [E, GW]
            m[name] = _sbuf_chunks(wt.astype(NP16))
        for name, bias, scale in (("bq", bq, SCALE), ("bk", bk, 1.0), ("bv", bv, 1.0)):
            m[name] = (np.asarray(bias, dtype=np.float32)[sl] * scale).astype(
                NP16).reshape(1, GW)
        m["maskt"] = _sbuf_chunks(np.ascontiguousarray(mask[b].T).astype(NP16))
        in_maps.append(m)

    _PROG_CACHE["in_maps"] = in_maps
    res = run_bass_kernel_spmd(nc, in_maps, list(range(N_CORES)))
    outs = res.results

    full = np.empty((B, S, E), dtype=np.float32)
    for c in range(N_CORES):
        b = c // 2
        g = c % 2
        full[b, :, GW * g:GW * g + GW] = outs[c]["out"]
    return full
